# revision 18
# baseline (speedup 1.0000x reference)
"""AttMaxPool2D (2x2 softmax-attention pooling) Trainium2 Bass kernel.

out[b, wo, ho, c] = sum_i p_i * exp(t*p_i) / sum_i exp(t*p_i)
over the 4 elements p_i of each 2x2 window of x[b, :, :, c] (softmax-
weighted pooling; jax.nn.softmax's max-subtraction cancels analytically).

Layout (per core; batch-parallel across 8 cores, 4 images each):
 - SBUF tiles hold [w:128(partitions), (h_chunk:16, c:128)(free)] slabs of
   one image; the HBM read per partition is 8KB contiguous.
 - e = exp(t*x) on ScalarE, pe = x*e split across VectorE/GpSimd.
 - h-pair (window row) sums sE=e_even+e_odd, sP=pe_even+pe_odd run on
   VectorE/GpSimd with strided APs (full fp32).
 - w-pair (window col) contraction runs on the PE as fp32 matmuls against a
   pair-sum 0/1 weight matrix; two consecutive h-chunks write the two
   partition halves of one [128,F] PSUM tile (via 128-column weights whose
   nonzero block sits at [j*64, j*64+64)) so the finals run full-width.
 - r = 1/den via the fast custom-DVE reciprocal (~51 ULP), out = num*r on
   VectorE.

variant="f32r" instead feeds un-presummed e/pe straight to float32r
matmuls (PSUM-accumulating the h-pairs): ~2x faster PE and less vector
work, but the f32r data path keeps only ~13 mantissa bits (measured
~2e-4 rel err), so it is not the default.
"""

import numpy as np
from contextlib import ExitStack

N_CORES = 8
B, W, H, C = 32, 128, 128, 128
BS = B // N_CORES            # images per core
HCHUNK = 16                  # h rows per slab
NPP = H // (2 * HCHUNK)      # psum iterations per image (h-chunk pairs)
WO, HO = W // 2, H // 2
FREE = HCHUNK * C            # slab free size (2048 f32)
PFREE = (HCHUNK // 2) * C    # psum free size (1024 f32)

# rows of each slab's pe-multiply done on GpSimd (of HCHUNK)
GP_PE_ROWS_F32 = 0
GP_PE_ROWS_F32R = 8


def _build(temp: float, reps: int = 1, variant: str = "fp32",
           dma_only: bool = False, no_pe: bool = False,
           gp_rows_ovr: int = None, direct_den: bool = False,
           hchunk: int = HCHUNK, sp_gp: bool = False,
           xbufs: int = 4, ebufs: int = 3, pebufs: int = 3, sbufs: int = 3,
           psbufs: int = None):
    import concourse.bacc as bacc
    import concourse.tile as tile
    from concourse import mybir

    f32 = mybir.dt.float32
    f32r = mybir.dt.float32r
    use_f32r = variant == "f32r"
    edt = f32r if use_f32r else f32

    free = hchunk * C
    pfree = (hchunk // 2) * C
    npp = H // (2 * hchunk)
    nq = pfree // 512
    if psbufs is None:
        psbufs = max(1, 8 // (2 * (pfree // 512 * 1)))
        psbufs = min(psbufs, 2)

    nc = bacc.Bacc("TRN2", target_bir_lowering=False, debug=False,
                   num_devices=N_CORES)
    x_ap = nc.dram_tensor("x", [BS, W, H, C], f32, kind="ExternalInput").ap()
    w_ap = nc.dram_tensor("wmat", [2, W, 128], edt,
                          kind="ExternalInput").ap()
    out_ap = nc.dram_tensor("out", [BS, WO, HO, C], f32,
                            kind="ExternalOutput").ap()

    with tile.TileContext(nc) as tc:
        with ExitStack() as ctx:
            wpool = ctx.enter_context(tc.tile_pool(name="w", bufs=1))
            xpool = ctx.enter_context(tc.tile_pool(name="x", bufs=xbufs))
            epool = ctx.enter_context(tc.tile_pool(name="e", bufs=ebufs))
            pepool = ctx.enter_context(tc.tile_pool(name="pe", bufs=pebufs))
            spool = ctx.enter_context(tc.tile_pool(name="s", bufs=sbufs))
            rpool = ctx.enter_context(tc.tile_pool(name="r", bufs=2))
            opool = ctx.enter_context(tc.tile_pool(name="o", bufs=2))
            pspool = ctx.enter_context(
                tc.tile_pool(name="ps", bufs=psbufs, space="PSUM"))

            wm = wpool.tile([W, 256], edt)
            nc.sync.dma_start(wm[:, 0:128], w_ap[0])
            nc.sync.dma_start(wm[:, 128:256], w_ap[1])

            gp_rows = GP_PE_ROWS_F32R if use_f32r else GP_PE_ROWS_F32
            if gp_rows_ovr is not None:
                gp_rows = gp_rows_ovr
            for _rep in range(reps):
                for b in range(BS):
                    for pp in range(npp):
                        den_ps = pspool.tile([128, pfree], f32)
                        num_ps = pspool.tile([128, pfree], f32)
                        for j2 in range(2):
                            hp = 2 * pp + j2
                            t3 = xpool.tile([128, free], f32, tag="t",
                                            name="t3").rearrange(
                                "p (h c) -> p h c", h=hchunk)
                            eng = nc.sync if (hp % 2 == 0) else nc.scalar
                            eng.dma_start(
                                t3,
                                x_ap[b, :, hp * hchunk:(hp + 1) * hchunk, :])
                            if dma_only:
                                continue
                            e3 = epool.tile([128, free], edt, tag="e",
                                            name="e3").rearrange(
                                "p (h c) -> p h c", h=hchunk)
                            nc.scalar.activation(
                                e3, t3, mybir.ActivationFunctionType.Exp,
                                scale=float(temp))
                            pe3 = pepool.tile([128, free], edt, tag="pe",
                                              name="pe3").rearrange(
                                "p (h c) -> p h c", h=hchunk)
                            k = hchunk - gp_rows
                            nc.vector.tensor_mul(
                                pe3[:, :k, :], t3[:, :k, :], e3[:, :k, :])
                            if gp_rows:
                                nc.gpsimd.tensor_mul(
                                    pe3[:, k:, :], t3[:, k:, :], e3[:, k:, :])
                            wm_j = wm[:, j2 * 128:(j2 + 1) * 128]
                            if use_f32r:
                                for q in range(nq):
                                    for dh in range(2):
                                        h0 = q * 8 + dh
                                        h1 = q * 8 + 8
                                        ps_sl = (slice(0, 128),
                                                 slice(q * 512,
                                                       (q + 1) * 512))
                                        st = (j2 == 0 and dh == 0)
                                        sp = (j2 == 1 and dh == 1)
                                        nc.tensor.matmul(
                                            den_ps[ps_sl], wm_j,
                                            e3[:, h0:h1:2, :],
                                            start=st, stop=sp)
                                        nc.tensor.matmul(
                                            num_ps[ps_sl], wm_j,
                                            pe3[:, h0:h1:2, :],
                                            start=st, stop=sp)
                            else:
                                sE = spool.tile([128, pfree], f32, tag="sE",
                                                name="sE").rearrange(
                                    "p (h c) -> p h c", h=hchunk // 2)
                                sP = spool.tile([128, pfree], f32, tag="sP",
                                                name="sP").rearrange(
                                    "p (h c) -> p h c", h=hchunk // 2)
                                if not direct_den:
                                    nc.vector.tensor_add(
                                        sE, e3[:, 0::2, :], e3[:, 1::2, :])
                                if sp_gp:
                                    nc.gpsimd.tensor_add(
                                        sP, pe3[:, 0::2, :], pe3[:, 1::2, :])
                                else:
                                    nc.vector.tensor_add(
                                        sP, pe3[:, 0::2, :], pe3[:, 1::2, :])
                                if no_pe:
                                    ho0 = hp * (hchunk // 2)
                                    nc.sync.dma_start(
                                        out_ap[b, :,
                                               ho0:ho0 + hchunk // 2, :],
                                        sE[0:64, :, :])
                                    continue
                                for q in range(nq):
                                    ps_sl = (slice(0, 128),
                                             slice(q * 512, (q + 1) * 512))
                                    q0, q1 = q * 4, (q + 1) * 4
                                    if direct_den:
                                        for dh in range(2):
                                            h0 = q * 8 + dh
                                            h1 = q * 8 + 8
                                            nc.tensor.matmul(
                                                den_ps[ps_sl], wm_j,
                                                e3[:, h0:h1:2, :],
                                                start=(j2 == 0 and dh == 0),
                                                stop=(j2 == 1 and dh == 1))
                                    else:
                                        nc.tensor.matmul(
                                            den_ps[ps_sl], wm_j,
                                            sE[:, q0:q1, :],
                                            start=(j2 == 0), stop=(j2 == 1))
                                    nc.tensor.matmul(
                                        num_ps[ps_sl], wm_j, sP[:, q0:q1, :],
                                        start=(j2 == 0), stop=(j2 == 1))
                        if no_pe:
                            continue
                        if dma_only:
                            for j2 in range(2):
                                ho0 = pp * hchunk + j2 * (hchunk // 2)
                                nc.sync.dma_start(
                                    out_ap[b, :, ho0:ho0 + hchunk // 2, :],
                                    t3[j2 * 64:(j2 + 1) * 64,
                                       0:hchunk // 2, :])
                            continue
                        r = rpool.tile([128, pfree], f32)
                        nc.vector.reciprocal_approx_fast(r[:], den_ps[:])
                        o = opool.tile([128, pfree], f32)
                        nc.vector.tensor_mul(o[:], num_ps[:], r[:])
                        o3 = o.rearrange("p (h c) -> p h c", h=hchunk // 2)
                        for j2 in range(2):
                            ho0 = pp * hchunk + j2 * (hchunk // 2)
                            nc.sync.dma_start(
                                out_ap[b, :, ho0:ho0 + hchunk // 2, :],
                                o3[j2 * 64:(j2 + 1) * 64, :, :])
    nc.compile()
    return nc


def _wmat() -> np.ndarray:
    w = np.zeros((2, W, 128), dtype=np.float32)
    for j in range(2):
        w[j, np.arange(W), j * 64 + np.arange(W) // 2] = 1.0
    return w


def kernel(x: np.ndarray, temperature: np.ndarray) -> np.ndarray:
    from concourse.bass_utils import run_bass_kernel_spmd

    x = np.ascontiguousarray(np.asarray(x, dtype=np.float32))
    temp = float(np.asarray(temperature, dtype=np.float32).reshape(-1)[0])
    nc = _build(temp, reps=1, variant="fp32")
    wmat = _wmat()
    shards = np.split(x, N_CORES, axis=0)
    in_maps = [{"x": s, "wmat": wmat} for s in shards]
    last_exc = None
    for _attempt in range(2):
        try:
            res = run_bass_kernel_spmd(nc, in_maps,
                                       core_ids=list(range(N_CORES)))
            break
        except Exception as exc:  # one retry in case of a wedged device
            last_exc = exc
    else:
        raise last_exc
    out = np.concatenate([res.results[i]["out"] for i in range(N_CORES)],
                         axis=0)
    return out.astype(np.float32)
